# revision 15
# baseline (speedup 1.0000x reference)
"""TRN2 Bass kernel for the E2E DRO module.

Math (per scenario t, vmapped over 2048 scenarios):
  Y_hat = X @ W.T + b;  ep = Y - Y_hat  (shared)
  64 projected-subgradient steps on (z in simplex, c free, lam >= 0) of the
  TV-DRO objective; returns (Z_star, Y_hat).

Gradient reformulation used on device (validated vs jax.grad):
  s = ep@z - c; u = s^2; umax = amax^2 (amax = max|s|); i* = argmax|s|
  m_i = |s_i| > t1,  t1 = sqrt(relu(umax - 2*lam));  cnt = sum(m); beta = 1 - cnt/n
  gz = (2/n) ep^T (m.s) + 2 beta s* ep[i*] - y_hat
  gc = -(2/n) 1^T (m.s)  - 2 beta s*
  glam = rho - 2 + 2 cnt / n
With epaug = [ep | -1] (n x 65) and z_aug = [z; c], all matmul terms come from
epaug^T @ (m~ . s~) where s~ = s / t1' (t1' = max(t1, 1e-20)) is computed by a
second matmul with rescaled z_aug so the mask threshold is the constant 1.0.

Sharding: data-parallel over scenarios, 256 per core; ep/W/b replicated.
"""

import os
import numpy as np
from contextlib import ExitStack

import concourse.bass as bass
import concourse.tile as tile
from concourse import bacc, mybir
from concourse.bass_utils import run_bass_kernel_spmd
from concourse.masks import make_identity

F32 = mybir.dt.float32
F32R = mybir.dt.float32r
U32 = mybir.dt.uint32
AL = mybir.AluOpType
AF = mybir.ActivationFunctionType
AX = mybir.AxisListType.X

N, NX, NY, E, P = 2048, 128, 64, 65, 128
NCH = N // P            # 16 obs chunks
NCORES = 8
TCORE = N // NCORES     # 256 scenarios per core
NT = TCORE // P         # 2 scenario tiles per core
N_ITER, LR0 = 64, 0.05
K_PROJ = 8              # Michelot rounds (converges in <=7 on this data)
TINY = 1e-20
NOBETA = bool(int(os.environ.get("NOBETA", "0")))
NOGATHER = bool(int(os.environ.get("NOGATHER", "0")))

_PROG_CACHE = {}


def _f32(x):
    return float(np.float32(x))


def _build(rho_val: float):
    nc = bacc.Bacc("TRN2", target_bir_lowering=False, debug=False,
                   num_devices=NCORES)
    X_d = nc.dram_tensor("X", [N, NX], F32, kind="ExternalInput")
    Y_d = nc.dram_tensor("Y", [N, NY], F32, kind="ExternalInput")
    Xs_d = nc.dram_tensor("Xs", [TCORE, NX], F32, kind="ExternalInput")
    W_d = nc.dram_tensor("W", [NY, NX], F32, kind="ExternalInput")
    b_d = nc.dram_tensor("b", [NY], F32, kind="ExternalInput")
    Z_d = nc.dram_tensor("Z", [TCORE, NY], F32, kind="ExternalOutput")
    YH_d = nc.dram_tensor("YH", [TCORE, NY], F32, kind="ExternalOutput")
    epaug_d = nc.dram_tensor("epaug_scratch", [N, E], F32)  # internal, for row gathers

    with tile.TileContext(nc) as tc, ExitStack() as ctx:
        gsem = ctx.enter_context(nc.semaphore("gather_sem"))
        gsem_val = [0]
        const = ctx.enter_context(tc.tile_pool(name="const", bufs=1))
        state = ctx.enter_context(tc.tile_pool(name="state", bufs=1))
        big = ctx.enter_context(tc.tile_pool(name="big", bufs=1))
        work = ctx.enter_context(tc.tile_pool(name="work", bufs=2))
        tiny = ctx.enter_context(tc.tile_pool(name="tiny", bufs=3))
        ps_big = ctx.enter_context(tc.tile_pool(name="ps_big", bufs=1, space="PSUM"))
        ps_sm = ctx.enter_context(tc.tile_pool(name="ps_sm", bufs=4, space="PSUM"))

        # ---------------- constants / precompute ----------------
        ident = const.tile([P, P], F32, tag="ident")
        make_identity(nc, ident[:])
        ones1 = const.tile([1, P], F32, tag="ones1")
        nc.vector.memset(ones1[:], 1.0)
        onescol_f = const.tile([P, 1], F32, tag="onescolf")
        nc.vector.memset(onescol_f[:], 1.0)

        # W -> Wt (128x x 64y)
        w_sb = tiny.tile([NY, NX], F32, tag="wsb")
        nc.gpsimd.dma_start(w_sb[:], W_d.ap())
        ps_w = ps_sm.tile([NX, NY], F32, tag="pss")
        nc.tensor.transpose(out=ps_w[:], in_=w_sb[:], identity=ident[0:NY, 0:NY])
        Wt = const.tile([NX, NY], F32, tag="Wt")
        nc.scalar.copy(Wt[:], ps_w[:])
        b_row = const.tile([1, NY], F32, tag="brow")
        nc.gpsimd.dma_start(b_row[:], b_d.ap().rearrange("(o d) -> o d", o=1))

        # X -> XT (128x x 2048obs)
        XT = big.tile([NX, N], F32, tag="XT")
        for k in range(NCH):
            xst = work.tile([P, NX], F32, tag="xstage")
            nc.gpsimd.dma_start(xst[:], X_d.ap()[k * P:(k + 1) * P, :])
            ps_x = ps_sm.tile([NX, P], F32, tag="pss")
            nc.tensor.transpose(out=ps_x[:], in_=xst[:], identity=ident[:])
            nc.scalar.copy(XT[:, k * P:(k + 1) * P], ps_x[:])
        # Xs -> XsT (128x x 256scen)
        XsT = const.tile([NX, TCORE], F32, tag="XsT")
        for m in range(NT):
            xst = work.tile([P, NX], F32, tag="xstage")
            nc.gpsimd.dma_start(xst[:], Xs_d.ap()[m * P:(m + 1) * P, :])
            ps_x = ps_sm.tile([NX, P], F32, tag="pss")
            nc.tensor.transpose(out=ps_x[:], in_=xst[:], identity=ident[:])
            nc.scalar.copy(XsT[:, m * P:(m + 1) * P], ps_x[:])

        # Y chunks (128 x 16*64)
        Ysb = big.tile([P, NCH, NY], F32, tag="Ysb")
        nc.gpsimd.dma_start(
            Ysb[:], Y_d.ap().rearrange("(k p) d -> p k d", p=P))

        # EPA (128 x 16 x 65) obs-layout epaug; epaugT (65 x 2048)
        EPA = big.tile([P, NCH, E], F32, tag="EPA")
        epaugT = big.tile([E, N], F32, tag="epaugT")
        for k in range(NCH):
            ps_yh = ps_sm.tile([P, NY], F32, tag="pss")
            nc.tensor.matmul(out=ps_yh[:], lhsT=XT[:, k * P:(k + 1) * P],
                             rhs=Wt[:], start=True, stop=False)
            nc.tensor.matmul(out=ps_yh[:], lhsT=ones1[:], rhs=b_row[:],
                             start=False, stop=True)
            nc.vector.tensor_tensor(out=EPA[:, k, 0:NY], in0=Ysb[:, k, :],
                                    in1=ps_yh[:], op=AL.subtract)
        nc.vector.memset(EPA[:, :, NY:E], -1.0)
        for k in range(NCH):
            ps_t = ps_sm.tile([E, P], F32, tag="pss")
            nc.tensor.transpose(out=ps_t[:], in_=EPA[:, k, :], identity=ident[:])
            nc.scalar.copy(epaugT[:, k * P:(k + 1) * P], ps_t[:])
        # epaug to DRAM for row gathers
        nc.gpsimd.dma_start(
            epaug_d.ap().rearrange("(k p) e -> p k e", p=P), EPA[:])

        # Y_hat slice (scen layout) + output + YHaug state
        YHaug = [state.tile([P, E], F32, tag=f"YHaug{m}", name=f"YHaug{m}") for m in range(NT)]
        for m in range(NT):
            ps_yh = ps_sm.tile([P, NY], F32, tag="pss")
            nc.tensor.matmul(out=ps_yh[:], lhsT=XsT[:, m * P:(m + 1) * P],
                             rhs=Wt[:], start=True, stop=False)
            nc.tensor.matmul(out=ps_yh[:], lhsT=ones1[:], rhs=b_row[:],
                             start=False, stop=True)
            nc.scalar.copy(YHaug[m][:, 0:NY], ps_yh[:])
            nc.vector.memset(YHaug[m][:, NY:E], 0.0)
            nc.gpsimd.dma_start(YH_d.ap()[m * P:(m + 1) * P, :],
                                YHaug[m][:, 0:NY])

        # c0 = mean(ep @ z0) over all obs, z0 uniform
        rowsums = tiny.tile([NY, 1], F32, tag="rows")
        nc.vector.tensor_reduce(out=rowsums[:], in_=epaugT[0:NY, :], axis=AX,
                                op=AL.add)
        ps_c0 = ps_sm.tile([1, 1], F32, tag="pss")
        nc.tensor.matmul(out=ps_c0[:], lhsT=rowsums[:], rhs=onescol_f[0:NY, :],
                         start=True, stop=True)
        c0s = tiny.tile([1, 1], F32, tag="c0s")
        nc.scalar.copy(c0s[:], ps_c0[:])
        ps_c0b = ps_sm.tile([P, 1], F32, tag="pss")
        nc.tensor.matmul(out=ps_c0b[:], lhsT=ones1[:], rhs=c0s[:],
                         start=True, stop=True)
        c0col = tiny.tile([P, 1], F32, tag="c0col")
        nc.vector.tensor_scalar(out=c0col[:], in0=ps_c0b[:],
                                scalar1=_f32(1.0 / (N * NY)), scalar2=None,
                                op0=AL.mult)

        # states
        ZaT = [state.tile([P, E], F32, tag=f"ZaT{m}", name=f"ZaT{m}") for m in range(NT)]
        lam2 = [state.tile([P, 1], F32, tag=f"lam2{m}", name=f"lam2{m}") for m in range(NT)]
        for m in range(NT):
            nc.vector.memset(ZaT[m][:, 0:NY], _f32(1.0 / NY))
            nc.vector.tensor_copy(ZaT[m][:, NY:E], c0col[:])
            nc.vector.memset(lam2[m][:], 2.0)

        # ---------------- iterations ----------------
        for t in range(N_ITER):
            lr = _f32(np.float32(LR0) / np.float32(np.sqrt(np.float32(t + 1.0))))
            Za_e = work.tile([E, TCORE], F32, tag="Za_e")
            Zt_e = work.tile([E, TCORE], F32, tag="Zt_e")
            amax = [None] * NT
            t1p = [None] * NT
            rt1 = [None] * NT
            EPG = [None] * NT
            sstar = [None] * NT
            cntc = [None] * NT

            for m in range(NT):
                # Za transpose -> (65 x 128)
                ps_z = ps_sm.tile([E, P], F32, tag="pss")
                nc.tensor.transpose(out=ps_z[:], in_=ZaT[m][:], identity=ident[:])
                nc.scalar.copy(Za_e[:, m * P:(m + 1) * P], ps_z[:])

                # scen-layout S matmul (128scen x 2048obs)
                ps_S = ps_big.tile([P, N], F32, tag="psbig")
                for j in range(4):
                    nc.tensor.matmul(
                        out=ps_S[:, j * 512:(j + 1) * 512],
                        lhsT=Za_e[:, m * P:(m + 1) * P],
                        rhs=epaugT[:, j * 512:(j + 1) * 512],
                        start=True, stop=True)
                # |S| in SBUF
                A_sb = work.tile([P, N], F32, tag=f"Asb{m}")
                nc.scalar.activation(A_sb[:], ps_S[:], AF.Abs)
                # amax + argmax
                amax[m] = tiny.tile([P, 1], F32, tag=f"amax{m}", name=f"amax{m}")
                nc.vector.tensor_reduce(out=amax[m][:], in_=A_sb[:], axis=AX,
                                        op=AL.max)
                amax8 = tiny.tile([P, 8], F32, tag=f"amax8{m}")
                nc.vector.tensor_copy(amax8[:], amax[m][:, 0:1].to_broadcast([P, 8]))
                EPG[m] = work.tile([P, E], F32, tag=f"EPG{m}", name=f"EPG{m}")
                if NOBETA:
                    nc.vector.memset(EPG[m][:], 0.0)
                else:
                    idx8 = tiny.tile([P, 8], U32, tag=f"idx8{m}")
                    nc.vector.max_index(idx8[:], amax8[:], A_sb[:])
                    if NOGATHER:
                        nc.vector.tensor_copy(EPG[m][:, 0:8],
                                              idx8[:].bitcast(F32))
                        nc.vector.memset(EPG[m][:], 0.0)
                    else:
                        with tc.tile_critical(name="gather"):
                            gsem_val[0] += 16
                            nc.gpsimd.indirect_dma_start(
                                out=EPG[m][:], out_offset=None, in_=epaug_d.ap(),
                                in_offset=bass.IndirectOffsetOnAxis(
                                    ap=idx8[:, 0:1], axis=0)).then_inc(gsem, 16)
                            nc.gpsimd.wait_ge(gsem, gsem_val[0])
                # t1 chain
                umax = tiny.tile([P, 1], F32, tag=f"umax{m}")
                nc.vector.tensor_tensor(out=umax[:], in0=amax[m][:],
                                        in1=amax[m][:], op=AL.mult)
                t1sq = tiny.tile([P, 1], F32, tag=f"t1sq{m}")
                nc.vector.tensor_tensor(out=t1sq[:], in0=umax[:], in1=lam2[m][:],
                                        op=AL.subtract)
                nc.vector.tensor_scalar(out=t1sq[:], in0=t1sq[:], scalar1=0.0,
                                        scalar2=None, op0=AL.max)
                t1 = tiny.tile([P, 1], F32, tag=f"t1_{m}")
                nc.scalar.activation(t1[:], t1sq[:], AF.Sqrt)
                t1p[m] = tiny.tile([P, 1], F32, tag=f"t1p{m}", name=f"t1p{m}")
                nc.vector.tensor_scalar(out=t1p[m][:], in0=t1[:], scalar1=TINY,
                                        scalar2=None, op0=AL.max)
                rt1[m] = tiny.tile([P, 1], F32, tag=f"rt1{m}", name=f"rt1{m}")
                nc.vector.reciprocal(rt1[m][:], t1p[m][:])
                mscr = work.tile([P, N], F32, tag="mscr", name="mscr")
                cntc[m] = tiny.tile([P, 1], F32, tag=f"cntc{m}", name=f"cntc{m}")
                nc.vector.tensor_scalar(out=mscr[:], in0=A_sb[:],
                                        scalar1=t1[:, 0:1], scalar2=None,
                                        op0=AL.is_gt, op1=AL.add,
                                        accum_out=cntc[m][:])
                # rescaled Z
                ZtT = tiny.tile([P, E], F32, tag=f"ZtT{m}")
                nc.vector.tensor_scalar(out=ZtT[:], in0=ZaT[m][:],
                                        scalar1=rt1[m][:, 0:1], scalar2=None,
                                        op0=AL.mult)
                ps_zt = ps_sm.tile([E, P], F32, tag="pss")
                nc.tensor.transpose(out=ps_zt[:], in_=ZtT[:], identity=ident[:])
                nc.scalar.copy(Zt_e[:, m * P:(m + 1) * P], ps_zt[:])
                # s* = <EPG, Za> per row
                dotscr = tiny.tile([P, E], F32, tag=f"dotscr{m}")
                nc.vector.tensor_tensor(out=dotscr[:], in0=EPG[m][:],
                                        in1=ZaT[m][:], op=AL.mult)
                sstar[m] = tiny.tile([P, 1], F32, tag=f"sstar{m}", name=f"sstar{m}")
                nc.vector.tensor_reduce(out=sstar[m][:], in_=dotscr[:], axis=AX,
                                        op=AL.add)

            # obs-layout rescaled S~, mask, masked values
            Mt = work.tile([P, NCH * TCORE], F32, tag="Mt")
            Wm = work.tile([P, NCH * TCORE], F32, tag="Wm")
            for g in range(2):
                ps_st = ps_big.tile([P, 8 * TCORE], F32, tag="psbig")
                for k8 in range(8):
                    k = 8 * g + k8
                    nc.tensor.matmul(
                        out=ps_st[:, k8 * TCORE:(k8 + 1) * TCORE],
                        lhsT=epaugT[:, k * P:(k + 1) * P],
                        rhs=Zt_e[:], start=True, stop=True)
                sl = slice(g * 8 * TCORE, (g + 1) * 8 * TCORE)
                ag = work.tile([P, 8 * TCORE], F32, tag="ag", name="ag")
                nc.scalar.activation(ag[:], ps_st[:], AF.Abs)
                nc.vector.tensor_scalar(out=Mt[:, sl], in0=ag[:],
                                        scalar1=1.0, scalar2=None,
                                        op0=AL.is_gt)
                nc.vector.tensor_tensor(out=Wm[:, sl], in0=Mt[:, sl],
                                        in1=ps_st[:], op=AL.mult)

            # contraction matmuls: R = epaug^T W (65 x 256); cnt = 1^T M (1 x 256)
            ps_R = ps_sm.tile([E, TCORE], F32, tag="pss")
            for k in range(NCH):
                nc.tensor.matmul(out=ps_R[:],
                                 lhsT=EPA[:, k, :],
                                 rhs=Wm[:, k * TCORE:(k + 1) * TCORE],
                                 start=(k == 0), stop=(k == NCH - 1))
            R_sb = work.tile([E, TCORE], F32, tag="Rsb")
            nc.scalar.copy(R_sb[:], ps_R[:])

            for m in range(NT):
                ps_RT = ps_sm.tile([P, E], F32, tag="pss")
                nc.tensor.transpose(out=ps_RT[:],
                                    in_=R_sb[:, m * P:(m + 1) * P],
                                    identity=ident[0:E, 0:E])
                # beta, kappas
                betaf = tiny.tile([P, 1], F32, tag=f"beta{m}")
                nc.vector.tensor_scalar(out=betaf[:], in0=cntc[m][:],
                                        scalar1=_f32(-1.0 / N), scalar2=1.0,
                                        op0=AL.mult, op1=AL.add)
                k2 = tiny.tile([P, 1], F32, tag=f"k2_{m}")
                nc.vector.tensor_tensor(out=k2[:], in0=betaf[:], in1=sstar[m][:],
                                        op=AL.mult)
                nc.vector.tensor_scalar(out=k2[:], in0=k2[:], scalar1=_f32(2.0 * lr),
                                        scalar2=None, op0=AL.mult)
                k1 = tiny.tile([P, 1], F32, tag=f"k1_{m}")
                nc.vector.tensor_scalar(out=k1[:], in0=t1p[m][:],
                                        scalar1=_f32(lr * 2.0 / N), scalar2=None,
                                        op0=AL.mult)
                # V = Za - k1*RT - k2*EPG + lr*YHaug
                a1 = tiny.tile([P, E], F32, tag=f"a1_{m}")
                nc.vector.tensor_scalar(out=a1[:], in0=ps_RT[:],
                                        scalar1=k1[:, 0:1], scalar2=None,
                                        op0=AL.mult)
                a2 = tiny.tile([P, E], F32, tag=f"a2_{m}")
                nc.vector.tensor_scalar(out=a2[:], in0=EPG[m][:],
                                        scalar1=k2[:, 0:1], scalar2=None,
                                        op0=AL.mult)
                yhl = tiny.tile([P, E], F32, tag=f"yhl{m}")
                nc.vector.tensor_scalar(out=yhl[:], in0=YHaug[m][:],
                                        scalar1=_f32(lr), scalar2=None,
                                        op0=AL.mult)
                V = work.tile([P, E], F32, tag=f"V{m}")
                nc.vector.tensor_tensor(out=V[:], in0=ZaT[m][:], in1=a1[:],
                                        op=AL.subtract)
                nc.vector.tensor_tensor(out=V[:], in0=V[:], in1=a2[:],
                                        op=AL.subtract)
                nc.vector.tensor_tensor(out=V[:], in0=V[:], in1=yhl[:],
                                        op=AL.add)
                # lambda2 update: lam2 = relu(lam2 - (4lr/N)cnt - 2lr(rho-2))
                g1 = tiny.tile([P, 1], F32, tag=f"g1_{m}")
                nc.vector.tensor_scalar(out=g1[:], in0=cntc[m][:],
                                        scalar1=_f32(4.0 * lr / N),
                                        scalar2=_f32(2.0 * lr * (rho_val - 2.0)),
                                        op0=AL.mult, op1=AL.add)
                nc.vector.tensor_tensor(out=lam2[m][:], in0=lam2[m][:], in1=g1[:],
                                        op=AL.subtract)
                nc.vector.tensor_scalar(out=lam2[m][:], in0=lam2[m][:],
                                        scalar1=0.0, scalar2=None, op0=AL.max)

                # ---- simplex projection (Michelot, K rounds) ----
                sv0 = tiny.tile([P, 1], F32, tag=f"sv0{m}")
                nc.vector.tensor_reduce(out=sv0[:], in_=V[:, 0:NY], axis=AX,
                                        op=AL.add)
                thn = tiny.tile([P, 1], F32, tag=f"thn{m}")
                nc.vector.tensor_scalar(out=thn[:], in0=sv0[:], scalar1=1.0,
                                        scalar2=_f32(-1.0 / NY),
                                        op0=AL.subtract, op1=AL.mult)
                thp = tiny.tile([P, 1], F32, tag=f"thp{m}")
                nc.vector.tensor_scalar(out=thp[:], in0=thn[:], scalar1=-1.0,
                                        scalar2=None, op0=AL.mult)
                pscr = tiny.tile([P, NY], F32, tag=f"pscr{m}")
                pscr2 = tiny.tile([P, NY], F32, tag=f"pscr2{m}")
                for r in range(K_PROJ):
                    sv = tiny.tile([P, 1], F32, tag=f"sv{m}")
                    nc.scalar.activation(pscr[:], V[:, 0:NY], AF.Relu,
                                         bias=thn[:, 0:1], scale=1.0,
                                         accum_out=sv[:])
                    cm = tiny.tile([P, 1], F32, tag=f"cm{m}")
                    nc.vector.tensor_scalar(out=pscr2[:], in0=V[:, 0:NY],
                                            scalar1=thp[:, 0:1], scalar2=None,
                                            op0=AL.is_gt, op1=AL.add,
                                            accum_out=cm[:])
                    rc = tiny.tile([P, 1], F32, tag=f"rc{m}")
                    nc.vector.reciprocal(rc[:], cm[:])
                    d = tiny.tile([P, 1], F32, tag=f"d{m}")
                    nc.vector.tensor_scalar(out=d[:], in0=sv[:], scalar1=1.0,
                                            scalar2=None, op0=AL.subtract)
                    nc.vector.tensor_tensor(out=d[:], in0=d[:], in1=rc[:],
                                            op=AL.mult)
                    nc.vector.tensor_tensor(out=thn[:], in0=thn[:], in1=d[:],
                                            op=AL.subtract)
                    nc.vector.tensor_scalar(out=thp[:], in0=thn[:], scalar1=-1.0,
                                            scalar2=None, op0=AL.mult)
                nc.scalar.activation(ZaT[m][:, 0:NY], V[:, 0:NY], AF.Relu,
                                     bias=thn[:, 0:1], scale=1.0)
                nc.vector.tensor_copy(ZaT[m][:, NY:E], V[:, NY:E])

        # ---------------- output ----------------
        for m in range(NT):
            nc.gpsimd.dma_start(Z_d.ap()[m * P:(m + 1) * P, :],
                                ZaT[m][:, 0:NY])

    nc.compile()
    return nc


def kernel(X, Y, rho, W, b):
    X = np.ascontiguousarray(X, np.float32)
    Y = np.ascontiguousarray(Y, np.float32)
    W = np.ascontiguousarray(W, np.float32)
    b = np.ascontiguousarray(b, np.float32)
    rho_val = float(np.float32(rho[0]))
    key = rho_val
    if key not in _PROG_CACHE:
        _PROG_CACHE[key] = _build(rho_val)
    nc = _PROG_CACHE[key]
    in_maps = [{
        "X": X, "Y": Y, "W": W, "b": b,
        "Xs": np.ascontiguousarray(X[c * TCORE:(c + 1) * TCORE]),
    } for c in range(NCORES)]
    res = run_bass_kernel_spmd(nc, in_maps, list(range(NCORES))).results
    Z = np.concatenate([res[c]["Z"] for c in range(NCORES)], 0)
    YH = np.concatenate([res[c]["YH"] for c in range(NCORES)], 0)
    return Z, YH
